# revision 17
# baseline (speedup 1.0000x reference)
"""Trainium2 Bass kernel for BertAdapterCapsuleMask.

Self-contained: takes full (unsharded) numpy inputs, shards across 8
NeuronCores, runs a fused Bass/Tile kernel per core, gathers the full output.

Key semantics note: the reference's `h_caps = vote.reshape(B, S, M*C)` is an
m-major flat reinterpret, so token n's 9 capsule inputs are vote values of
tokens ~3n from a single m-block — NOT batch-local.  We handle this by
computing the cheap part (semantic capsules -> squash -> routing priors,
~0.5% of FLOPs) exactly on the host, pre-scrambling priors into each core's
consumer "stream order" (rows (d, r, c), d = which-of-3-source-tokens), and
running the iterative routing + all heavy matmuls on device.  In stream
order the final vote tile IS h_caps in consumer layout, so the larger/adapter
matmuls consume it directly.

Device layout: feature dims on SBUF partitions, tokens on the free dim.
Linear layers are accumulating f32r matmuls (weights pre-transposed on host,
sigmoid gates folded into weights/biases); the routing's tiny (d, task, cap)
reductions/broadcasts are matmuls against small host-built selector matrices.
The causal task mask reduces routing to the first Teff = t+1 tasks exactly
(masked tasks have softmax weight exp(-10000) == 0 in fp32).
"""

import sys

sys.path.insert(0, "/opt/trn_rl_repo")
import numpy as np

B, S, H, A, T, C, M3 = 128, 128, 768, 2000, 10, 3, 3
NCORES = 8
NTOK = B * S                  # 16384 tokens total
NCT = NTOK // NCORES          # 2048 tokens per core
NCHUNK = 512                  # tokens per pipeline chunk (PSUM bank = 512 f32)
NCH = NCT // NCHUNK           # 4 chunks per core
APAD = 2048                   # A=2000 zero-padded to 16x128
AC = APAD // 128              # 16 a-chunks
HC = H // 128                 # 6 h-chunks
EPS = 1e-16
NV = M3 * C                   # 9 rows: (d, c)

_CACHE = {}


def _sel_shapes(Teff):
    NL = M3 * Teff
    NP = M3 * Teff * C
    return {
        "sq9to3": (NV, M3),      # sum squares of vote per d
        "exp3to9": (M3, NV),     # per-d scalar -> (d, c)
        "exp9toNP": (NV, NP),    # outputs (d,c) -> (d, r, c)
        "redNPtoNL": (NP, NL),   # sum over c: (d,r,c) -> (d,r)
        "expNLtoNP": (NL, NP),   # E (d,r) -> (d,r,c)
        "redNLto3": (NL, M3),    # sum over r: (d,r) -> d
        "redNPto9": (NP, NV),    # sum over r: (d,r,c) -> (d,c)
    }


def _build(Teff, repeat=1, loop_repeat=1):
    """Build + compile the per-core Bass program (shapes depend on Teff=t+1).

    repeat>1 unrolls the whole computation R times (timing builds only)."""
    import concourse.bacc as bacc
    import concourse.mybir as mybir
    import concourse.tile as tile

    f32 = mybir.dt.float32
    f32r = mybir.dt.float32r
    AF = mybir.ActivationFunctionType
    OP = mybir.AluOpType

    NL = M3 * Teff
    NP = M3 * Teff * C
    sel_shapes = _sel_shapes(Teff)

    nc = bacc.Bacc("TRN2", target_bir_lowering=False, debug=False)

    dx = nc.dram_tensor("xT", [H, NCT], f32r, kind="ExternalInput").ap()
    dw1 = nc.dram_tensor("w1T", [H, APAD], f32r, kind="ExternalInput").ap()
    dw2 = nc.dram_tensor("w2g", [APAD, H], f32r, kind="ExternalInput").ap()
    dlw9 = nc.dram_tensor("lw9", [NV, H], f32r, kind="ExternalInput").ap()
    dp54 = nc.dram_tensor("p54s", [NCH, NP, NCHUNK], f32, kind="ExternalInput").ap()
    dv0 = nc.dram_tensor("v0s", [NCH, NV, NCHUNK], f32, kind="ExternalInput").ap()
    dcon = nc.dram_tensor("consts", [128, 35], f32, kind="ExternalInput").ap()
    dsel = {
        k: nc.dram_tensor(k, list(v), f32r, kind="ExternalInput").ap()
        for k, v in sel_shapes.items()
    }
    dout = nc.dram_tensor("outT", [H, NCT], f32, kind="ExternalOutput").ap()

    with tile.TileContext(nc) as tc, \
         nc.allow_low_precision(reason="f32r tiles feed PE matmuls by design"):
        with tc.tile_pool(name="wp", bufs=1) as wp, \
             tc.tile_pool(name="px", bufs=2) as px, \
             tc.tile_pool(name="pout", bufs=1) as pout, \
             tc.tile_pool(name="ph1", bufs=1) as ph1, \
             tc.tile_pool(name="phT", bufs=1) as phT, \
             tc.tile_pool(name="prt", bufs=8) as prt, \
             tc.tile_pool(name="pp54", bufs=2) as pp54, \
             tc.tile_pool(name="pL", bufs=2) as pL, \
             tc.tile_pool(name="pg2", bufs=1) as pg2, \
             tc.tile_pool(name="psmm", bufs=2, space="PSUM") as psmm, \
             tc.tile_pool(name="psrt", bufs=4, space="PSUM") as psrt:

            # ---- persistent weight/constant loads -------------------------
            selt = {}
            for k, (p, m) in sel_shapes.items():
                tl = wp.tile([p, m], f32r, name=f"sel_{k}")
                nc.sync.dma_start(tl[:], dsel[k][:, :])
                selt[k] = tl
            cont = wp.tile([128, 35], f32, name="consts")
            nc.sync.dma_start(cont[:], dcon[:, :])
            lw9t = wp.tile([NV, H], f32r, name="lw9")
            nc.sync.dma_start(lw9t[:], dlw9[:, :])
            w1t = []
            for k in range(HC):
                tl = wp.tile([128, APAD], f32r, name=f"w1_{k}")
                nc.sync.dma_start(tl[:], dw1[k * 128:(k + 1) * 128, :])
                w1t.append(tl)
            w2t = []
            for a in range(AC):
                tl = wp.tile([128, H], f32r, name=f"w2_{a}")
                nc.sync.dma_start(tl[:], dw2[a * 128:(a + 1) * 128, :])
                w2t.append(tl)

            b1 = lambda a: cont[:, a:a + 1]            # noqa: E731
            b2 = lambda h: cont[:, 16 + h:17 + h]      # noqa: E731
            gf2 = lambda h: cont[:, 22 + h:23 + h]     # noqa: E731
            lb = lambda h: cont[:, 28 + h:29 + h]      # noqa: E731
            epsc = lambda n: cont[0:n, 34:35]          # noqa: E731

            def coef_block(name, sq_ps, ngrp):
                """squash coefficient from group sum-of-squares psum [ngrp, n]:
                coef = (sq+eps) / ((1+sq+eps) * sqrt(sq+eps)),  f32r tile."""
                sqr = prt.tile([ngrp, NCHUNK], f32, tag="rt", name=f"sqr_{name}")
                nc.scalar.activation(sqr[:], sq_ps[:], AF.Sqrt, bias=epsc(ngrp))
                den = prt.tile([ngrp, NCHUNK], f32, tag="rt", name=f"den_{name}")
                nc.vector.scalar_tensor_tensor(den[:], sq_ps[:], 1.0 + EPS, sqr[:],
                                               OP.add, OP.mult)
                rec = prt.tile([ngrp, NCHUNK], f32, tag="rt", name=f"rec_{name}")
                nc.vector.reciprocal(rec[:], den[:])
                coef = prt.tile([ngrp, NCHUNK], f32r, tag="rt", name=f"coef_{name}")
                nc.vector.scalar_tensor_tensor(coef[:], sq_ps[:], EPS, rec[:],
                                               OP.add, OP.mult)
                return coef

            def squash9(name, vote_src):
                """vote [NV, n] (sbuf) -> squashed outputs [NV, n] f32r."""
                vv = prt.tile([NV, NCHUNK], f32r, tag="rt", name=f"vv_{name}")
                nc.vector.tensor_mul(vv[:], vote_src[:], vote_src[:])
                sqm = psrt.tile([M3, NCHUNK], f32, tag="ps_rt", name=f"sqm_{name}")
                nc.tensor.matmul(sqm[:], selt["sq9to3"][:], vv[:], start=True, stop=True)
                coef = coef_block(name, sqm, M3)
                ce9 = psrt.tile([NV, NCHUNK], f32, tag="ps_rt", name=f"ce9_{name}")
                nc.tensor.matmul(ce9[:], selt["exp3to9"][:], coef[:], start=True, stop=True)
                outp = prt.tile([NV, NCHUNK], f32r, tag="rt", name=f"outp_{name}")
                nc.vector.tensor_mul(outp[:], vote_src[:], ce9[:])
                return outp

            def delta_block(name, outp, p54):
                """outputs [NV,n] -> delta-logits psum [NL, n]."""
                o54 = psrt.tile([NP, NCHUNK], f32, tag="ps_rt", name=f"o54_{name}")
                nc.tensor.matmul(o54[:], selt["exp9toNP"][:], outp[:], start=True, stop=True)
                prd = prt.tile([NP, NCHUNK], f32r, tag="rt", name=f"prd_{name}")
                nc.vector.tensor_mul(prd[:], p54[:], o54[:])
                dl = psrt.tile([NL, NCHUNK], f32, tag="ps_rt", name=f"dl_{name}")
                nc.tensor.matmul(dl[:], selt["redNPtoNL"][:], prd[:], start=True, stop=True)
                return dl

            def vote_block(name, e_tile, p54):
                """E = exp(logits) [NL,n] -> softmax-weighted vote [NV, n] f32r."""
                dn = psrt.tile([M3, NCHUNK], f32, tag="ps_rt", name=f"dn_{name}")
                nc.tensor.matmul(dn[:], selt["redNLto3"][:], e_tile[:], start=True, stop=True)
                rcd = prt.tile([M3, NCHUNK], f32r, tag="rt", name=f"rcd_{name}")
                nc.vector.reciprocal(rcd[:], dn[:])
                e54 = psrt.tile([NP, NCHUNK], f32, tag="ps_rt", name=f"e54_{name}")
                nc.tensor.matmul(e54[:], selt["expNLtoNP"][:], e_tile[:], start=True, stop=True)
                pre = prt.tile([NP, NCHUNK], f32r, tag="rt", name=f"pre_{name}")
                nc.vector.tensor_mul(pre[:], p54[:], e54[:])
                vu = psrt.tile([NV, NCHUNK], f32, tag="ps_rt", name=f"vu_{name}")
                nc.tensor.matmul(vu[:], selt["redNPto9"][:], pre[:], start=True, stop=True)
                vusb = prt.tile([NV, NCHUNK], f32, tag="rt", name=f"vusb_{name}")
                nc.vector.tensor_copy(vusb[:], vu[:])
                r9 = psrt.tile([NV, NCHUNK], f32, tag="ps_rt", name=f"r9_{name}")
                nc.tensor.matmul(r9[:], selt["exp3to9"][:], rcd[:], start=True, stop=True)
                vt = prt.tile([NV, NCHUNK], f32r, tag="rt", name=f"vt_{name}")
                nc.vector.tensor_mul(vt[:], vusb[:], r9[:])
                return vt

            # ---- per-chunk pipeline --------------------------------------
            import contextlib
            loop_cm = (tc.For_i(0, loop_repeat, 1) if loop_repeat > 1
                       else contextlib.nullcontext())
            with loop_cm:
              for rr in range(repeat):
               for c0 in range(NCH):
                c = rr * NCH + c0
                cs = c0 * NCHUNK
                xt = []
                for k in range(HC):
                    tl = px.tile([128, NCHUNK], f32r, tag=f"x{k}", name=f"x{k}_{c}")
                    nc.sync.dma_start(tl[:], dx[k * 128:(k + 1) * 128, cs:cs + NCHUNK])
                    xt.append(tl)
                p54 = pp54.tile([NP, NCHUNK], f32, tag="p54", name=f"p54_{c}")
                nc.sync.dma_start(p54[:], dp54[c0, :, :])
                v0sb = prt.tile([NV, NCHUNK], f32, tag="rt", name=f"v0sb_{c}")
                nc.sync.dma_start(v0sb[:], dv0[c0, :, :])

                # -- routing iter 0: outputs0 from uniform vote0, L1 = delta0
                outp0 = squash9(f"{c}_0", v0sb)
                dl0 = delta_block(f"{c}_0", outp0, p54)
                L1 = pL.tile([NL, NCHUNK], f32, tag="L", name=f"L1_{c}")
                nc.vector.tensor_copy(L1[:], dl0[:])

                # -- routing iter 1
                E1 = prt.tile([NL, NCHUNK], f32r, tag="rt", name=f"E1_{c}")
                nc.scalar.activation(E1[:], L1[:], AF.Exp)
                vt1 = vote_block(f"{c}_1", E1, p54)
                outp1 = squash9(f"{c}_1", vt1)
                dl1 = delta_block(f"{c}_1", outp1, p54)
                L2 = pL.tile([NL, NCHUNK], f32, tag="L", name=f"L2_{c}")
                nc.vector.tensor_add(L2[:], L1[:], dl1[:])

                # -- routing iter 2 (final): vt2 is h_caps in consumer layout
                E2 = prt.tile([NL, NCHUNK], f32r, tag="rt", name=f"E2_{c}")
                nc.scalar.activation(E2[:], L2[:], AF.Exp)
                vt2 = vote_block(f"{c}_2", E2, p54)

                # -- larger linear (gates folded) + residual: hT = lw9.T@vote + lb + x
                hT = []
                for h in range(HC):
                    pl = psmm.tile([128, NCHUNK], f32, tag="mm", name=f"pl_{c}_{h}")
                    nc.tensor.matmul(pl[:], lw9t[:, h * 128:(h + 1) * 128], vt2[:],
                                     start=True, stop=True)
                    ht = phT.tile([128, NCHUNK], f32r, tag=f"hT{h}", name=f"hT{h}_{c}")
                    nc.vector.scalar_tensor_tensor(ht[:], pl[:], lb(h), xt[h][:],
                                                   OP.add, OP.add)
                    hT.append(ht)

                # -- adapter mm1: h1 = gelu(w1.T @ hT + b1) (gfc1 folded into w2g)
                h1 = []
                for a in range(AC):
                    p1 = psmm.tile([128, NCHUNK], f32, tag="mm", name=f"p1_{c}_{a}")
                    for k in range(HC):
                        nc.tensor.matmul(p1[:], w1t[k][:, a * 128:(a + 1) * 128],
                                         hT[k][:], start=(k == 0), stop=(k == HC - 1))
                    ht1 = ph1.tile([128, NCHUNK], f32r, tag=f"h1_{a}", name=f"h1_{a}_{c}")
                    nc.scalar.activation(ht1[:], p1[:], AF.Gelu, bias=b1(a))
                    h1.append(ht1)

                # -- adapter mm2 + gelu + gate + residual -> out
                for h in range(HC):
                    p2 = psmm.tile([128, NCHUNK], f32, tag="mm", name=f"p2_{c}_{h}")
                    for a in range(AC):
                        nc.tensor.matmul(p2[:], w2t[a][:, h * 128:(h + 1) * 128],
                                         h1[a][:], start=(a == 0), stop=(a == AC - 1))
                    g2 = pg2.tile([128, NCHUNK], f32, tag="g2", name=f"g2_{c}_{h}")
                    nc.scalar.activation(g2[:], p2[:], AF.Gelu, bias=b2(h))
                    ot = pout.tile([128, NCHUNK], f32, tag=f"o{h}", name=f"o{h}_{c}")
                    nc.vector.scalar_tensor_tensor(ot[:], g2[:], gf2(h), xt[h][:],
                                                   OP.mult, OP.add)
                    nc.sync.dma_start(dout[h * 128:(h + 1) * 128, cs:cs + NCHUNK], ot[:])

    nc.compile()
    return nc


def _sigmoid(v):
    return 1.0 / (1.0 + np.exp(-v.astype(np.float64)))


def _prep_inputs(x, t, s, fc1_w, fc1_b, fc2_w, fc2_b, efc1, efc2,
                 sem_w, sem_b, route_weights, larger_w, larger_b, elarger):
    t = int(np.asarray(t).item())
    sv = float(np.asarray(s).reshape(-1)[0])
    Teff = t + 1
    NL = M3 * Teff
    NP = M3 * Teff * C

    f = np.float32
    gfc1 = _sigmoid(sv * np.asarray(efc1)[t]).astype(f)          # [A]
    gfc2 = _sigmoid(sv * np.asarray(efc2)[t]).astype(f)          # [H]
    glarger = _sigmoid(sv * np.asarray(elarger)[t]).astype(f)    # [H]

    w1T = np.zeros((H, APAD), f)
    w1T[:, :A] = np.asarray(fc1_w, f).T
    w2g = np.zeros((APAD, H), f)
    w2g[:A] = np.asarray(fc2_w, f).T * gfc1[:, None]
    lw9 = np.ascontiguousarray((np.asarray(larger_w, f) * glarger[:, None]).T)  # [9, H]
    lb = (np.asarray(larger_b, f) * glarger).astype(f)           # [H]

    b1p = np.zeros(APAD, f)
    b1p[:A] = np.asarray(fc1_b, f)
    consts = np.zeros((128, 35), f)
    consts[:, 0:16] = b1p.reshape(16, 128).T
    consts[:, 16:22] = np.asarray(fc2_b, f).reshape(6, 128).T
    consts[:, 22:28] = gfc2.reshape(6, 128).T
    consts[:, 28:34] = lb.reshape(6, 128).T
    consts[:, 34] = EPS

    # ---- host: semantic capsules -> squash -> priors (exact, f64) --------
    x2 = np.asarray(x, f).reshape(NTOK, H).astype(np.float64)
    semw = np.asarray(sem_w, np.float64).transpose(2, 1, 0).reshape(H, C * T)
    semb = np.asarray(sem_b, np.float64).T.reshape(C * T)
    sem = x2 @ semw + semb                                       # [N, 30] (c*T+t)
    g = sem.reshape(NTOK, C, T)
    sq = np.sum(g * g, axis=-1, keepdims=True) + EPS
    v = (sq / (1.0 + sq)) * g / np.sqrt(sq)                      # squash over t
    x5 = v.reshape(NTOK, T, C)
    rw = np.asarray(route_weights, np.float64)
    pri = np.einsum("nrc,mrcd->mnrd", x5[:, :Teff], rw[:, :Teff])  # [3,N,Teff,3]
    v0f = pri.mean(axis=2)                                       # [3, N, 3]

    # selector matrices (lhsT layout [K, M])
    sq9to3 = np.zeros((NV, M3), f)
    exp3to9 = np.zeros((M3, NV), f)
    for d in range(M3):
        for cc in range(C):
            sq9to3[d * C + cc, d] = 1.0
            exp3to9[d, d * C + cc] = 1.0
    exp9toNP = np.zeros((NV, NP), f)
    redNPtoNL = np.zeros((NP, NL), f)
    expNLtoNP = np.zeros((NL, NP), f)
    redNLto3 = np.zeros((NL, M3), f)
    redNPto9 = np.zeros((NP, NV), f)
    for d in range(M3):
        for r in range(Teff):
            redNLto3[d * Teff + r, d] = 1.0
            for cc in range(C):
                q = d * Teff * C + r * C + cc
                exp9toNP[d * C + cc, q] = 1.0
                redNPtoNL[q, d * Teff + r] = 1.0
                expNLtoNP[d * Teff + r, q] = 1.0
                redNPto9[q, d * C + cc] = 1.0

    const_map = {
        "w1T": w1T, "w2g": w2g, "lw9": lw9, "consts": consts,
        "sq9to3": sq9to3, "exp3to9": exp3to9, "exp9toNP": exp9toNP,
        "redNPtoNL": redNPtoNL, "expNLtoNP": expNLtoNP, "redNLto3": redNLto3,
        "redNPto9": redNPto9,
    }

    # stream-order scramble per core: consumer (ca, nl2, j=3d+c) pulls vote of
    # (m, n') with  q = ci*3*NCT + 3*(ca*512+nl2) + d;  m = q//NTOK, n' = q%NTOK
    nl2 = np.arange(NCH * NCHUNK)                                # [2048]
    dd = np.arange(M3)
    x32 = np.asarray(x, f).reshape(NTOK, H)
    in_maps = []
    for ci in range(NCORES):
        q = ci * 3 * NCT + 3 * nl2[None, :] + dd[:, None]        # [3, 2048]
        m_idx = q // NTOK
        n_idx = q % NTOK
        blk = pri[m_idx, n_idx]                                  # [3, 2048, Teff, 3]
        p54s = blk.transpose(0, 2, 3, 1).reshape(NP, NCH, NCHUNK)
        p54s = np.ascontiguousarray(p54s.transpose(1, 0, 2)).astype(f)
        vblk = v0f[m_idx, n_idx]                                 # [3, 2048, 3]
        v0s = vblk.transpose(0, 2, 1).reshape(NV, NCH, NCHUNK)
        v0s = np.ascontiguousarray(v0s.transpose(1, 0, 2)).astype(f)
        xT = np.ascontiguousarray(x32[ci * NCT:(ci + 1) * NCT].T)  # [H, NCT]
        m = dict(const_map)
        m["xT"] = xT
        m["p54s"] = p54s
        m["v0s"] = v0s
        in_maps.append(m)
    return Teff, in_maps


def run_sharded(trace=False, **inputs):
    """Run on hardware; returns (full_output [B,S,H] f32, exec_time_ns|None)."""
    from concourse.bass_utils import run_bass_kernel_spmd

    Teff, in_maps = _prep_inputs(**inputs)
    if Teff not in _CACHE:
        _CACHE[Teff] = _build(Teff)
    nc = _CACHE[Teff]
    res = run_bass_kernel_spmd(nc, in_maps, list(range(NCORES)), trace=trace)
    outs = [res.results[ci]["outT"] for ci in range(NCORES)]      # each [H, NCT]
    full = np.empty((NTOK, H), np.float32)
    for ci in range(NCORES):
        full[ci * NCT:(ci + 1) * NCT] = outs[ci].T
    return full.reshape(B, S, H), res.exec_time_ns


def kernel(**inputs):
    out, _ = run_sharded(trace=False, **inputs)
    return out


# revision 21
# speedup vs baseline: 1.8011x; 1.8011x over previous
"""Trainium2 Bass kernel for BertAdapterCapsuleMask.

Self-contained: takes full (unsharded) numpy inputs, shards across 8
NeuronCores, runs a fused Bass/Tile kernel per core, gathers the full output.

Key semantics note: the reference's `h_caps = vote.reshape(B, S, M*C)` is an
m-major flat reinterpret, so token n's 9 capsule inputs are vote values of
tokens ~3n from a single m-block — NOT batch-local.  We handle this by
computing the cheap part (semantic capsules -> squash -> routing priors,
~0.5% of FLOPs) exactly on the host, pre-scrambling priors into each core's
consumer "stream order" (rows (d, r, c), d = which-of-3-source-tokens), and
running the iterative routing + all heavy matmuls on device.  In stream
order the final vote tile IS h_caps in consumer layout, so the larger/adapter
matmuls consume it directly.

Device layout: feature dims on SBUF partitions, tokens on the free dim.
Linear layers are accumulating f32r matmuls (weights pre-transposed on host,
sigmoid gates folded into weights/biases); the routing's tiny (d, task, cap)
reductions/broadcasts are matmuls against small host-built selector matrices.
The causal task mask reduces routing to the first Teff = t+1 tasks exactly
(masked tasks have softmax weight exp(-10000) == 0 in fp32).
"""

import sys

sys.path.insert(0, "/opt/trn_rl_repo")
import numpy as np

B, S, H, A, T, C, M3 = 128, 128, 768, 2000, 10, 3, 3
NCORES = 8
NTOK = B * S                  # 16384 tokens total
NCT = NTOK // NCORES          # 2048 tokens per core
NCHUNK = 512                  # tokens per pipeline chunk (PSUM bank = 512 f32)
NCH = NCT // NCHUNK           # 4 chunks per core
APAD = 2048                   # A=2000 zero-padded to 16x128
AC = APAD // 128              # 16 a-chunks
HC = H // 128                 # 6 h-chunks
EPS = 1e-16
NV = M3 * C                   # 9 rows: (d, c)

_CACHE = {}
BF16_MM = False   # bf16 weights/activations for the two big adapter matmuls


def _sel_shapes(Teff):
    NL = M3 * Teff
    NP = M3 * Teff * C
    return {
        "sq9to3": (NV, M3),      # sum squares of vote per d
        "exp3to9": (M3, NV),     # per-d scalar -> (d, c)
        "exp9toNP": (NV, NP),    # outputs (d,c) -> (d, r, c)
        "redNPtoNL": (NP, NL),   # sum over c: (d,r,c) -> (d,r)
        "expNLtoNP": (NL, NP),   # E (d,r) -> (d,r,c)
        "redNLto3": (NL, M3),    # sum over r: (d,r) -> d
        "redNPto9": (NP, NV),    # sum over r: (d,r,c) -> (d,c)
    }


def _build(Teff, repeat=1, loop_repeat=1, bf16_mm=None):
    """Build + compile the per-core Bass program (shapes depend on Teff=t+1).

    repeat>1 unrolls the whole computation R times (timing builds only)."""
    import concourse.bacc as bacc
    import concourse.mybir as mybir
    import concourse.tile as tile

    f32 = mybir.dt.float32
    f32r = mybir.dt.float32r
    if bf16_mm is None:
        bf16_mm = BF16_MM
    wdt = mybir.dt.bfloat16 if bf16_mm else f32r
    AF = mybir.ActivationFunctionType
    OP = mybir.AluOpType

    NL = M3 * Teff
    NP = M3 * Teff * C
    sel_shapes = _sel_shapes(Teff)

    nc = bacc.Bacc("TRN2", target_bir_lowering=False, debug=False)

    dx = nc.dram_tensor("xT", [H, NCT], f32r, kind="ExternalInput").ap()
    dw1 = nc.dram_tensor("w1T", [H, APAD], wdt, kind="ExternalInput").ap()
    dw2 = nc.dram_tensor("w2g", [APAD, H], wdt, kind="ExternalInput").ap()
    dlw9 = nc.dram_tensor("lw9", [NV, H], f32r, kind="ExternalInput").ap()
    dp54 = nc.dram_tensor("p54s", [NCH, NP, NCHUNK], f32, kind="ExternalInput").ap()
    dv0 = nc.dram_tensor("v0s", [NCH, NV, NCHUNK], f32, kind="ExternalInput").ap()
    dcon = nc.dram_tensor("consts", [128, 35], f32, kind="ExternalInput").ap()
    dsel = {
        k: nc.dram_tensor(k, list(v), f32r, kind="ExternalInput").ap()
        for k, v in sel_shapes.items()
    }
    dout = nc.dram_tensor("outT", [H, NCT], f32, kind="ExternalOutput").ap()

    with tile.TileContext(nc) as tc, \
         nc.allow_low_precision(reason="f32r tiles feed PE matmuls by design"):
        with tc.tile_pool(name="wp", bufs=1) as wp, \
             tc.tile_pool(name="px", bufs=2) as px, \
             tc.tile_pool(name="pout", bufs=1) as pout, \
             tc.tile_pool(name="ph1", bufs=1) as ph1, \
             tc.tile_pool(name="phT", bufs=1) as phT, \
             tc.tile_pool(name="prt", bufs=8) as prt, \
             tc.tile_pool(name="pp54", bufs=2) as pp54, \
             tc.tile_pool(name="pL", bufs=2) as pL, \
             tc.tile_pool(name="pg2", bufs=1) as pg2, \
             tc.tile_pool(name="psmm", bufs=3, space="PSUM") as psmm, \
             tc.tile_pool(name="psrt", bufs=4, space="PSUM") as psrt:

            # ---- small constant loads (selectors, consts, lw9) ------------
            selt = {}
            for k, (pp, mm) in sel_shapes.items():
                tl = wp.tile([pp, mm], f32r, name=f"sel_{k}")
                nc.sync.dma_start(tl[:], dsel[k][:, :])
                selt[k] = tl
            cont = wp.tile([128, 35], f32, name="consts")
            nc.sync.dma_start(cont[:], dcon[:, :])
            lw9t = wp.tile([NV, H], f32r, name="lw9")
            nc.sync.dma_start(lw9t[:], dlw9[:, :])
            w1t = [wp.tile([128, APAD], wdt, name=f"w1_{k}") for k in range(HC)]
            w2t = [wp.tile([128, H], wdt, name=f"w2_{a}") for a in range(AC)]

            b1 = lambda a: cont[:, a:a + 1]            # noqa: E731
            b2 = lambda h: cont[:, 16 + h:17 + h]      # noqa: E731
            gf2 = lambda h: cont[:, 22 + h:23 + h]     # noqa: E731
            lb = lambda h: cont[:, 28 + h:29 + h]      # noqa: E731
            epsc = lambda n: cont[0:n, 34:35]          # noqa: E731

            def coef_chain(nm, sq_ps, ngrp):
                """squash coefficient from group sum-of-squares psum [ngrp,n]:
                coef = (sq+eps) / ((1+sq+eps) * sqrt(sq+eps)),  f32r tile."""
                sqr = prt.tile([ngrp, NCHUNK], f32, tag="rt", name=f"sqr_{nm}")
                nc.scalar.activation(sqr[:], sq_ps[:], AF.Sqrt, bias=epsc(ngrp))
                den = prt.tile([ngrp, NCHUNK], f32, tag="rt", name=f"den_{nm}")
                nc.vector.scalar_tensor_tensor(den[:], sq_ps[:], 1.0 + EPS, sqr[:],
                                               OP.add, OP.mult)
                rec = prt.tile([ngrp, NCHUNK], f32, tag="rt", name=f"rec_{nm}")
                nc.vector.reciprocal(rec[:], den[:])
                coef = prt.tile([ngrp, NCHUNK], f32r, tag="rt", name=f"coef_{nm}")
                nc.vector.scalar_tensor_tensor(coef[:], sq_ps[:], EPS, rec[:],
                                               OP.add, OP.mult)
                return coef

            state = {}

            def routing_units(nm, c0):
                """Routing chain for chunk c0, one yield per PE-anchored unit.

                Stores vt2 (h_caps tile) and the chunk's x tiles in state[nm]."""
                cs = c0 * NCHUNK
                xt = []
                for k in range(HC):
                    tl = px.tile([128, NCHUNK], f32r, tag=f"x{k}", name=f"x{k}_{nm}")
                    nc.sync.dma_start(tl[:], dx[k * 128:(k + 1) * 128, cs:cs + NCHUNK])
                    xt.append(tl)
                p54 = pp54.tile([NP, NCHUNK], f32, tag="p54", name=f"p54_{nm}")
                nc.sync.dma_start(p54[:], dp54[c0, :, :])
                v0sb = prt.tile([NV, NCHUNK], f32, tag="rt", name=f"v0sb_{nm}")
                nc.sync.dma_start(v0sb[:], dv0[c0, :, :])
                yield

                def squash9_units(snm, vote_src):
                    vv = prt.tile([NV, NCHUNK], f32r, tag="rt", name=f"vv_{snm}")
                    nc.vector.tensor_mul(vv[:], vote_src[:], vote_src[:])
                    yield
                    sqm = psrt.tile([M3, NCHUNK], f32, tag="ps_rt", name=f"sqm_{snm}")
                    nc.tensor.matmul(sqm[:], selt["sq9to3"][:], vv[:], start=True, stop=True)
                    coef = coef_chain(snm, sqm, M3)
                    yield
                    ce9 = psrt.tile([NV, NCHUNK], f32, tag="ps_rt", name=f"ce9_{snm}")
                    nc.tensor.matmul(ce9[:], selt["exp3to9"][:], coef[:], start=True, stop=True)
                    outp = prt.tile([NV, NCHUNK], f32r, tag="rt", name=f"outp_{snm}")
                    nc.vector.tensor_mul(outp[:], vote_src[:], ce9[:])
                    state[f"outp_{snm}"] = outp

                def delta_units(snm, outp):
                    o54 = psrt.tile([NP, NCHUNK], f32, tag="ps_rt", name=f"o54_{snm}")
                    nc.tensor.matmul(o54[:], selt["exp9toNP"][:], outp[:], start=True, stop=True)
                    prd = prt.tile([NP, NCHUNK], f32r, tag="rt", name=f"prd_{snm}")
                    nc.vector.tensor_mul(prd[:], p54[:], o54[:])
                    yield
                    dl = psrt.tile([NL, NCHUNK], f32, tag="ps_rt", name=f"dl_{snm}")
                    nc.tensor.matmul(dl[:], selt["redNPtoNL"][:], prd[:], start=True, stop=True)
                    state[f"dl_{snm}"] = dl

                def vote_units(snm, e_tile):
                    dn = psrt.tile([M3, NCHUNK], f32, tag="ps_rt", name=f"dn_{snm}")
                    nc.tensor.matmul(dn[:], selt["redNLto3"][:], e_tile[:], start=True, stop=True)
                    rcd = prt.tile([M3, NCHUNK], f32r, tag="rt", name=f"rcd_{snm}")
                    nc.vector.reciprocal(rcd[:], dn[:])
                    yield
                    e54 = psrt.tile([NP, NCHUNK], f32, tag="ps_rt", name=f"e54_{snm}")
                    nc.tensor.matmul(e54[:], selt["expNLtoNP"][:], e_tile[:], start=True, stop=True)
                    pre = prt.tile([NP, NCHUNK], f32r, tag="rt", name=f"pre_{snm}")
                    nc.vector.tensor_mul(pre[:], p54[:], e54[:])
                    yield
                    vu = psrt.tile([NV, NCHUNK], f32, tag="ps_rt", name=f"vu_{snm}")
                    nc.tensor.matmul(vu[:], selt["redNPto9"][:], pre[:], start=True, stop=True)
                    vusb = prt.tile([NV, NCHUNK], f32, tag="rt", name=f"vusb_{snm}")
                    nc.vector.tensor_copy(vusb[:], vu[:])
                    yield
                    r9 = psrt.tile([NV, NCHUNK], f32, tag="ps_rt", name=f"r9_{snm}")
                    nc.tensor.matmul(r9[:], selt["exp3to9"][:], rcd[:], start=True, stop=True)
                    vt = prt.tile([NV, NCHUNK], f32r, tag="rt", name=f"vt_{snm}")
                    nc.vector.tensor_mul(vt[:], vusb[:], r9[:])
                    state[f"vt_{snm}"] = vt

                # iter 0
                yield from squash9_units(f"{nm}_0", v0sb)
                yield
                yield from delta_units(f"{nm}_0", state[f"outp_{nm}_0"])
                yield
                L1 = pL.tile([NL, NCHUNK], f32, tag="L", name=f"L1_{nm}")
                nc.vector.tensor_copy(L1[:], state[f"dl_{nm}_0"][:])
                E1 = prt.tile([NL, NCHUNK], f32r, tag="rt", name=f"E1_{nm}")
                nc.scalar.activation(E1[:], L1[:], AF.Exp)
                # iter 1
                yield from vote_units(f"{nm}_1", E1)
                yield
                vt1 = state[f"vt_{nm}_1"]
                yield from squash9_units(f"{nm}_1s", vt1)
                yield
                yield from delta_units(f"{nm}_1", state[f"outp_{nm}_1s"])
                yield
                L2 = pL.tile([NL, NCHUNK], f32, tag="L", name=f"L2_{nm}")
                nc.vector.tensor_add(L2[:], L1[:], state[f"dl_{nm}_1"][:])
                E2 = prt.tile([NL, NCHUNK], f32r, tag="rt", name=f"E2_{nm}")
                nc.scalar.activation(E2[:], L2[:], AF.Exp)
                # iter 2 (final)
                yield from vote_units(f"{nm}_2", E2)
                state[f"vt2_{nm}"] = state[f"vt_{nm}_2"]
                state[f"xt_{nm}"] = xt

            def big_units(nm, c0):
                """larger + adapter matmuls for chunk c0, one yield per psum group."""
                cs = c0 * NCHUNK
                vt2 = state[f"vt2_{nm}"]
                xt = state[f"xt_{nm}"]
                hT = []
                for h in range(HC):
                    pl = psmm.tile([128, NCHUNK], f32, tag="mm", name=f"pl_{nm}_{h}")
                    nc.tensor.matmul(pl[:], lw9t[:, h * 128:(h + 1) * 128], vt2[:],
                                     start=True, stop=True)
                    ht = phT.tile([128, NCHUNK], wdt, tag=f"hT{h}", name=f"hT{h}_{nm}")
                    nc.vector.scalar_tensor_tensor(ht[:], pl[:], lb(h), xt[h][:],
                                                   OP.add, OP.add)
                    hT.append(ht)
                    if h % 2 == 1:
                        yield
                h1 = []
                for a in range(AC):
                    p1 = psmm.tile([128, NCHUNK], f32, tag="mm", name=f"p1_{nm}_{a}")
                    for k in range(HC):
                        nc.tensor.matmul(p1[:], w1t[k][:, a * 128:(a + 1) * 128],
                                         hT[k][:], start=(k == 0), stop=(k == HC - 1))
                    ht1 = ph1.tile([128, NCHUNK], wdt, tag=f"h1_{a}", name=f"h1_{a}_{nm}")
                    nc.scalar.activation(ht1[:], p1[:], AF.Gelu, bias=b1(a))
                    h1.append(ht1)
                    yield
                for h in range(HC):
                    p2 = psmm.tile([128, NCHUNK], f32, tag="mm", name=f"p2_{nm}_{h}")
                    for a in range(AC):
                        nc.tensor.matmul(p2[:], w2t[a][:, h * 128:(h + 1) * 128],
                                         h1[a][:], start=(a == 0), stop=(a == AC - 1))
                    g2 = pg2.tile([128, NCHUNK], f32, tag="g2", name=f"g2_{nm}_{h}")
                    nc.scalar.activation(g2[:], p2[:], AF.Gelu, bias=b2(h))
                    ot = pout.tile([128, NCHUNK], f32, tag=f"o{h}", name=f"o{h}_{nm}")
                    nc.vector.scalar_tensor_tensor(ot[:], g2[:], gf2(h), xt[h][:],
                                                   OP.mult, OP.add)
                    nc.sync.dma_start(dout[h * 128:(h + 1) * 128, cs:cs + NCHUNK], ot[:])
                    yield

            def drain(gen):
                for _ in gen:
                    pass

            # ---- pipelined schedule: routing(c+1) interleaves into big(c) --
            import contextlib
            loop_cm = (tc.For_i(0, loop_repeat, 1) if loop_repeat > 1
                       else contextlib.nullcontext())
            with loop_cm:
                for rr in range(repeat):
                    drain(routing_units(f"{rr}_0", 0))
                    if rr == 0:
                        # weight DMAs issued after chunk-0 routing's DMAs so the
                        # routing chain (and its x tiles) aren't queued behind
                        # 12 MB of weights; w1 in quarters so mm1 starts early.
                        for q in range(4):
                            for k in range(HC):
                                nc.sync.dma_start(
                                    w1t[k][:, q * 512:(q + 1) * 512],
                                    dw1[k * 128:(k + 1) * 128, q * 512:(q + 1) * 512])
                        for a in range(AC):
                            nc.sync.dma_start(w2t[a][:], dw2[a * 128:(a + 1) * 128, :])
                    for c0 in range(NCH):
                        nm = f"{rr}_{c0}"
                        rgen = (routing_units(f"{rr}_{c0 + 1}", c0 + 1)
                                if c0 + 1 < NCH else None)
                        for _ in big_units(nm, c0):
                            if rgen is not None:
                                next(rgen, None)
                        if rgen is not None:
                            drain(rgen)

    nc.compile()
    return nc


def _sigmoid(v):
    return 1.0 / (1.0 + np.exp(-v.astype(np.float64)))


def _prep_inputs(x, t, s, fc1_w, fc1_b, fc2_w, fc2_b, efc1, efc2,
                 sem_w, sem_b, route_weights, larger_w, larger_b, elarger):
    t = int(np.asarray(t).item())
    sv = float(np.asarray(s).reshape(-1)[0])
    Teff = t + 1
    NL = M3 * Teff
    NP = M3 * Teff * C

    f = np.float32
    gfc1 = _sigmoid(sv * np.asarray(efc1)[t]).astype(f)          # [A]
    gfc2 = _sigmoid(sv * np.asarray(efc2)[t]).astype(f)          # [H]
    glarger = _sigmoid(sv * np.asarray(elarger)[t]).astype(f)    # [H]

    w1T = np.zeros((H, APAD), f)
    w1T[:, :A] = np.asarray(fc1_w, f).T
    w2g = np.zeros((APAD, H), f)
    w2g[:A] = np.asarray(fc2_w, f).T * gfc1[:, None]
    if BF16_MM:
        import ml_dtypes
        w1T = w1T.astype(ml_dtypes.bfloat16)
        w2g = w2g.astype(ml_dtypes.bfloat16)
    lw9 = np.ascontiguousarray((np.asarray(larger_w, f) * glarger[:, None]).T)  # [9, H]
    lb = (np.asarray(larger_b, f) * glarger).astype(f)           # [H]

    b1p = np.zeros(APAD, f)
    b1p[:A] = np.asarray(fc1_b, f)
    consts = np.zeros((128, 35), f)
    consts[:, 0:16] = b1p.reshape(16, 128).T
    consts[:, 16:22] = np.asarray(fc2_b, f).reshape(6, 128).T
    consts[:, 22:28] = gfc2.reshape(6, 128).T
    consts[:, 28:34] = lb.reshape(6, 128).T
    consts[:, 34] = EPS

    # ---- host: semantic capsules -> squash -> priors (exact, f64) --------
    x2 = np.asarray(x, f).reshape(NTOK, H).astype(np.float64)
    semw = np.asarray(sem_w, np.float64).transpose(2, 1, 0).reshape(H, C * T)
    semb = np.asarray(sem_b, np.float64).T.reshape(C * T)
    sem = x2 @ semw + semb                                       # [N, 30] (c*T+t)
    g = sem.reshape(NTOK, C, T)
    sq = np.sum(g * g, axis=-1, keepdims=True) + EPS
    v = (sq / (1.0 + sq)) * g / np.sqrt(sq)                      # squash over t
    x5 = v.reshape(NTOK, T, C)
    rw = np.asarray(route_weights, np.float64)
    pri = np.einsum("nrc,mrcd->mnrd", x5[:, :Teff], rw[:, :Teff])  # [3,N,Teff,3]
    v0f = pri.mean(axis=2)                                       # [3, N, 3]

    # selector matrices (lhsT layout [K, M])
    sq9to3 = np.zeros((NV, M3), f)
    exp3to9 = np.zeros((M3, NV), f)
    for d in range(M3):
        for cc in range(C):
            sq9to3[d * C + cc, d] = 1.0
            exp3to9[d, d * C + cc] = 1.0
    exp9toNP = np.zeros((NV, NP), f)
    redNPtoNL = np.zeros((NP, NL), f)
    expNLtoNP = np.zeros((NL, NP), f)
    redNLto3 = np.zeros((NL, M3), f)
    redNPto9 = np.zeros((NP, NV), f)
    for d in range(M3):
        for r in range(Teff):
            redNLto3[d * Teff + r, d] = 1.0
            for cc in range(C):
                q = d * Teff * C + r * C + cc
                exp9toNP[d * C + cc, q] = 1.0
                redNPtoNL[q, d * Teff + r] = 1.0
                expNLtoNP[d * Teff + r, q] = 1.0
                redNPto9[q, d * C + cc] = 1.0

    const_map = {
        "w1T": w1T, "w2g": w2g, "lw9": lw9, "consts": consts,
        "sq9to3": sq9to3, "exp3to9": exp3to9, "exp9toNP": exp9toNP,
        "redNPtoNL": redNPtoNL, "expNLtoNP": expNLtoNP, "redNLto3": redNLto3,
        "redNPto9": redNPto9,
    }

    # stream-order scramble per core: consumer (ca, nl2, j=3d+c) pulls vote of
    # (m, n') with  q = ci*3*NCT + 3*(ca*512+nl2) + d;  m = q//NTOK, n' = q%NTOK
    nl2 = np.arange(NCH * NCHUNK)                                # [2048]
    dd = np.arange(M3)
    x32 = np.asarray(x, f).reshape(NTOK, H)
    in_maps = []
    for ci in range(NCORES):
        q = ci * 3 * NCT + 3 * nl2[None, :] + dd[:, None]        # [3, 2048]
        m_idx = q // NTOK
        n_idx = q % NTOK
        blk = pri[m_idx, n_idx]                                  # [3, 2048, Teff, 3]
        p54s = blk.transpose(0, 2, 3, 1).reshape(NP, NCH, NCHUNK)
        p54s = np.ascontiguousarray(p54s.transpose(1, 0, 2)).astype(f)
        vblk = v0f[m_idx, n_idx]                                 # [3, 2048, 3]
        v0s = vblk.transpose(0, 2, 1).reshape(NV, NCH, NCHUNK)
        v0s = np.ascontiguousarray(v0s.transpose(1, 0, 2)).astype(f)
        xT = np.ascontiguousarray(x32[ci * NCT:(ci + 1) * NCT].T)  # [H, NCT]
        m = dict(const_map)
        m["xT"] = xT
        m["p54s"] = p54s
        m["v0s"] = v0s
        in_maps.append(m)
    return Teff, in_maps


def run_sharded(trace=False, **inputs):
    """Run on hardware; returns (full_output [B,S,H] f32, exec_time_ns|None)."""
    from concourse.bass_utils import run_bass_kernel_spmd

    Teff, in_maps = _prep_inputs(**inputs)
    if Teff not in _CACHE:
        _CACHE[Teff] = _build(Teff)
    nc = _CACHE[Teff]
    res = run_bass_kernel_spmd(nc, in_maps, list(range(NCORES)), trace=trace)
    outs = [res.results[ci]["outT"] for ci in range(NCORES)]      # each [H, NCT]
    full = np.empty((NTOK, H), np.float32)
    for ci in range(NCORES):
        full[ci * NCT:(ci + 1) * NCT] = outs[ci].T
    return full.reshape(B, S, H), res.exec_time_ns


def kernel(**inputs):
    out, _ = run_sharded(trace=False, **inputs)
    return out
